# revision 38
# baseline (speedup 1.0000x reference)
"""GATv2 message-passing kernel for 8 Trainium2 NeuronCores (Bass/Tile), v3.

Strategy (edge parallelism over receiver-sorted edges), changes vs v2:
  * mish is never materialized.  Using mish(x) = x*(1 - 2/((1+e^x)^2+1)),
    logits = A^T x - 2 A^T (x*rc) with rc = 1/((1+u)^2+1): one ACT Exp, one
    ACT Copy, two 4x-rate DVE tensor_scalar ops (fused add+pow) and one DVE
    mult per chunk; the -2 folds into a second logits matmul against -2A.
    This removes the exp/ln/tanh table thrash entirely (every ACT func used
    lives in the exp_and_others set), so the act-table patch and the G_ILV
    position interleave are gone.
  * The division in the mish tail runs as two registered custom-DVE ops
    (den, then a seeded 1-Newton reciprocal fused with the x multiply).
  * The softmax weight expansion (per-head -> per-head-dim) runs as an ACT
    broadcast-copy so stage 3's multiply has no broadcast operand (2x DVE);
    MV_ACT_FRAC balances ACT vs DVE if needed.
  * Two race fixes over v2 (both verified by the CoreSim race detector):
    per-PSUM-bank stop bits on the es^T accumulation chains, and
    SWDGE DMASW semaphore lanes partitioned by queue (see the
    tile_sem_assignment patch below) — the stock round-robin let queue-0
    indirect-DMA completions update a semaphore still owned by a queue-3
    gather, which intermittently let consumers run before gathers landed.
  * Phase A is sharded: each core computes h for its 1/8 of the nodes and
    one HBM->HBM AllGather replicates it (PHASE_A_MODE).  The replicated
    phase A cost ~156us of critical path (DMA-bound nfT2 read + h write);
    the sharded version costs ~20us + the collective.  Repeat-slope HW
    measurement: allgather 327us vs replicated 422us per pipeline rep
    (v2 baseline: 710us).
  * Everything else (receiver-sorted edge plan, fp8 one-hot DMA, bf16
    gathers, identity-transposes, deferred output stores) as in v2.
"""

import ml_dtypes
import numpy as np

import concourse.bass as bass
import concourse.bacc as bacc
import concourse.tile as tile
from concourse import mybir
from concourse.bass_utils import run_bass_kernel_spmd

F32 = mybir.dt.float32
BF16 = mybir.dt.bfloat16
FP8 = mybir.dt.float8e4
I32 = mybir.dt.int32
AF = mybir.ActivationFunctionType
OP = mybir.AluOpType
BF = ml_dtypes.bfloat16
F8 = ml_dtypes.float8_e4m3fn

# --------------------------------------------------------------- custom DVE
# Two single-instruction DVE ops for the mish tail (registered through the
# same OPS table the in-tree ops use):
#   ANT_GAT_DEN:    den = (in0 + s0)^2 + s0            (s0 = 1)
#   ANT_RECIP1_MUL: out = in1 * recip1(in0), recip1 = bitwise-NOT exponent
#     seed * s1, one Newton-ish polish (s2 - x*y0); with the minimax pair
#     s1=-0.2354980, s2=2.0017330 the result is 1/in0 to ~0.17% — far inside
#     this kernel's tolerance.  (DVE has no divide/pow ALU op, ACT's
#     reciprocal table lives in a different set than Exp, and the bit-exact
#     InstReciprocal runs at ~6 cycles/elem, so this is the only fast path.)
import concourse.dve_ops as _dve_ops
from concourse.dve_spec import (
    AluOp as _AluOp, Bin as _Bin, C0 as _C0, C1 as _C1, C2 as _C2,
    Spec as _Spec, Src0 as _Src0, Src1 as _Src1, lower as _dve_lower,
    sq as _dve_sq, _has_src1 as _dve_has_src1,
)
from concourse.dve_uop import DveOpSpec as _DveOpSpec


def _register_dve_op(name, spec, perf_en=True):
    for o in _dve_ops.OPS:
        if o.name == name:
            return o
    row = _dve_ops._CUSTOM_DVE_ROW_BASE + len(_dve_ops.OPS)
    assert row < 0x20, "custom-DVE opcode rows exhausted"
    _dve_ops._SUB_OPCODE_FOR_NAME[name] = row
    shas, pens = {}, {}
    for ver in ("v3", "v4"):
        try:
            s = _DveOpSpec(name=name, opcode=row,
                           uops=_dve_lower(spec, ver=ver),
                           rd1_en=_dve_has_src1(spec))
            shas[ver] = s.sha(ver)
            pens[ver] = perf_en
        except Exception:
            pass
    op = _dve_ops.DveOp(name, spec, subdim=False, uops_sha=shas,
                        perf_en=pens)
    _dve_ops.OPS.append(op)
    _dve_ops.CUSTOM_DVE_SPECS[name] = spec
    return op


_GAT_DEN = _register_dve_op("ANT_GAT_DEN", _Spec(
    body=_dve_sq(_Src0 + _C0) + _C0,
    reference=lambda in0, in1, s0, s1, imm2: (in0 + s0) * (in0 + s0) + s0,
))


def _ref_recip1_mul(in0, in1, s0, s1, imm2):
    not_x = (~in0.astype(np.float32).view(np.int32)).view(np.float32)
    y0 = not_x * s1
    return y0 * (imm2 - in0 * y0) * in1


_nx = _Bin(_AluOp.BITWISE_NOT, _Src0, _Src0)
_y0 = _nx * _C1
_RECIP1_MUL = _register_dve_op("ANT_RECIP1_MUL", _Spec(
    body=(_y0 * (_C2 - _Src0 * _y0)) * _Src1,
    reference=_ref_recip1_mul,
))
RECIP1_C = dict(s0=0.0, s1=-0.2354980, imm2=2.0017330)


def _ref_mish1_mul(in0, in1, s0, s1, imm2):
    not_x = (~in0.astype(np.float32).view(np.int32)).view(np.float32)
    y0 = not_x * s1
    y1 = y0 * (imm2 - in0 * y0)
    return y1 * (in0 - s0) * in1


# out = in1 * (in0 - 2) * recip1(in0): with in0 = den = (1+u)^2+1 and
# in1 = x (read straight from PSUM) this is mish(x) = x*(u^2+2u)/den in one
# DVE pass — no ACT Copy of x and a single logits matmul.
_MISH1_MUL = _register_dve_op("ANT_MISH1_MUL", _Spec(
    body=((_y0 * (_C2 - _Src0 * _y0)) * (_Src0 - _C0)) * _Src1,
    reference=_ref_mish1_mul,
))
MISH1_C = dict(s0=2.0, s1=-0.2354980, imm2=2.0017330)

# ------------------------------------------------------- SWDGE sem lanes
# Tile's sem assignment round-robins every software-DGE DMA over the 8
# DMASW semaphore lanes regardless of which SWDGE queue the op runs on, so
# a lane's semaphore can be owned by queue-3 gather completions while a
# queue-0 indirect DMA updates it — the queues then miscount completions
# and a consumer can run before its data lands (CoreSim flags exactly
# this: "update semaphore ... locked to SWDGE queue 3").  Partition the
# lanes by queue instead: queue q uses lanes {2q, 2q+1} only.
import concourse.tile_sem_assignment as _tsa

if not getattr(_tsa.TileClockTick._assign_tick, "_ant_qlanes", False):
    _orig_assign_tick = _tsa.TileClockTick._assign_tick

    def _assign_tick_qlanes(self, inst):
        if (isinstance(inst, _tsa.DMAInst)
                and not isinstance(inst,
                                   _tsa.bass_isa.UserSyncedRemoteDMADescs)
                and inst.engine == mybir.EngineType.Pool):
            qn = getattr(inst, "queue_num", None)
            if not isinstance(qn, int):
                qn = 0  # qPoolDynamic indirect DMAs ride queue 0
            cnt = self.__dict__.setdefault("_ant_qlane_cnt", {})
            k = cnt.get(qn, 0)
            cnt[qn] = k + 1
            self.next_sw_dma_idx = (2 * qn + (k & 1)) % 8
        return _orig_assign_tick(self, inst)

    _assign_tick_qlanes._ant_qlanes = True
    _tsa.TileClockTick._assign_tick = _assign_tick_qlanes

N_NODES = 50000
N_EDGES = 800000
IN_DIM = 256
EDGE_DIM = 64
EMBED = 128
HEADS = 8
HEAD_DIM = EMBED // HEADS
P = 128
NCORES = 8
CHUNK_G = 8  # groups (of 128 edges) per processing chunk
PAD_RLOC = 200.0  # sentinel receiver-local id for padding edges (> 127)
# "replicated": every core computes all of h (no collectives).
# "allgather": each core computes its 1/8 shard of h, then one HBM->HBM
# AllGather replicates it (saves ~63us of nfT2 DMA + 37us PE + 44us ACT per
# core, at the price of the collective's latency).
PHASE_A_MODE = "allgather"


# ---------------------------------------------------------------- host plan

def _plan(receivers, senders, n_nodes, ncores):
    """Sort edges by receiver, then by sender within each 128-node receiver
    block (so gathers use monotone addresses and fit int16 index windows);
    deal blocks to cores balanced by edge count; pad every (core, position)
    to a common group count; pick per-(position, chunk) gather base offsets
    shared by all cores."""
    order = np.argsort(receivers, kind="stable").astype(np.int64)
    r_s = receivers[order].astype(np.int64)
    nb = -(-n_nodes // P)
    npos = -(-nb // ncores)
    nb_pad = npos * ncores
    n_pad = nb_pad * P
    cnt = np.bincount(r_s // P, minlength=nb_pad).astype(np.int64)
    estart = np.zeros(nb_pad, np.int64)
    estart[1:] = np.cumsum(cnt)[:-1]
    # sender-sort within each receiver block
    for b in range(nb_pad):
        e0, c = int(estart[b]), int(cnt[b])
        if c > 1:
            seg = order[e0:e0 + c]
            order[e0:e0 + c] = seg[np.argsort(senders[seg], kind="stable")]
    r_s = receivers[order].astype(np.int64)
    gcnt = np.maximum(-(-cnt // P), 1)
    deal = np.argsort(-gcnt, kind="stable")
    blocks = deal.reshape(npos, ncores)  # blocks[pos, core] -> block id
    gpos = gcnt[blocks].max(axis=1)      # groups per position (same all cores)
    goff = np.zeros(npos, np.int64)
    goff[1:] = np.cumsum(gpos)[:-1]
    # per-(position, chunk) gather windows, uniform across cores.  A gather
    # descriptor is (group offset within position, n groups, base row); when
    # a full chunk's sender window overflows int16, split it in halves.
    def _window(pos, glo, ghi):
        lo, hi = np.iinfo(np.int64).max, 0
        for core in range(ncores):
            b = int(blocks[pos, core])
            e0, c = int(estart[b]), int(cnt[b])
            sc = senders[order[e0:e0 + c]].astype(np.int64)
            part = sc[glo * P:min(ghi * P, c)]
            if part.size:
                lo = min(lo, int(part.min()))
                hi = max(hi, int(part.max()))
        if lo > hi:
            return 0, 0
        return lo, hi

    gdesc = []
    for pos in range(npos):
        gp = int(gpos[pos])
        descs = []
        for ch in range(-(-gp // CHUNK_G)):
            g_lo = ch * CHUNK_G
            g_hi = min(g_lo + CHUNK_G, gp)
            lo, hi = _window(pos, g_lo, g_hi)
            if hi - lo < 32768:
                descs.append((g_lo, g_hi - g_lo, lo))
            else:
                mid = g_lo + (g_hi - g_lo + 1) // 2
                for a_, b_ in ((g_lo, mid), (mid, g_hi)):
                    lo, hi = _window(pos, a_, b_)
                    assert hi - lo < 32768, \
                        f"gather window overflow at pos {pos}: {hi - lo}"
                    descs.append((a_, b_ - a_, lo))
        gdesc.append(descs)
    return dict(order=order, r_s=r_s, cnt=cnt, estart=estart, blocks=blocks,
                gpos=gpos, goff=goff, gtot=int(gpos.sum()),
                ecap=int(gpos.sum()) * P, npos=npos, nb_pad=nb_pad,
                gdesc=gdesc, n_pad=n_pad)


def _host_inputs(plan, node_features, edge_features, W_kernel, W_bias,
                 We_kernel, We_bias, a, senders):
    """Build the per-core input maps (all numpy, no math beyond transposes)."""
    npos, gtot, ecap = plan["npos"], plan["gtot"], plan["ecap"]
    n_pad = plan["nb_pad"] * P
    n_nodes, in_dim = node_features.shape
    heads, head_dim = a.shape
    embed = heads * head_dim
    edge_dim = edge_features.shape[1]
    # interleaved nfT: per sweep of HW_ nodes, k-rows 0:128 then 128:256
    nfT = np.zeros((in_dim, n_pad), np.float32)
    nfT[:, :n_nodes] = node_features.T

    def _interleave(cols, HW_):
        n = cols.shape[1]
        out = np.zeros((P, 2 * n), BF)
        for s in range(n // HW_):
            out[:, 2 * s * HW_:2 * s * HW_ + HW_] = \
                cols[0:P, s * HW_:(s + 1) * HW_]
            out[:, 2 * s * HW_ + HW_:2 * (s + 1) * HW_] = \
                cols[P:2 * P, s * HW_:(s + 1) * HW_]
        return out

    if PHASE_A_MODE == "allgather":
        SH = n_pad // NCORES
        nfT2_cores = [_interleave(nfT[:, c * SH:(c + 1) * SH], 7 * P)
                      for c in range(NCORES)]
        nfT2 = None
    else:
        HW_ = 8 * P
        nfT2 = np.zeros((P, 2 * n_pad), BF)
        for s in range(n_pad // HW_):
            nfT2[:, 2 * s * HW_:2 * s * HW_ + HW_] = \
                nfT[0:P, s * HW_:(s + 1) * HW_]
            nfT2[:, 2 * s * HW_ + HW_:2 * (s + 1) * HW_] = \
                nfT[P:2 * P, s * HW_:(s + 1) * HW_]
    We_aug = np.concatenate(
        [We_kernel, (We_bias + 2.0 * W_bias)[None, :]], axis=0
    ).astype(BF)
    A_blk = np.zeros((embed, heads), np.float32)
    for h in range(heads):
        A_blk[h * head_dim:(h + 1) * head_dim, h] = a[h]
    Wb_rep = np.tile(W_bias[None, :], (P, 1)).astype(np.float32)
    identity = np.eye(P, dtype=BF)

    efT_all = np.ascontiguousarray(edge_features[plan["order"]].T).astype(BF)
    s_sorted = senders[plan["order"]].astype(np.int32)
    rloc_all = (plan["r_s"] - (plan["r_s"] // P) * P).astype(np.int64)

    qrange = np.arange(P, dtype=np.int64)
    shared = {
        "W": W_kernel.astype(BF), "We_aug": We_aug,
        "A_blk": A_blk.astype(BF), "Abm2": (-2.0 * A_blk).astype(BF),
        "Wb_rep": Wb_rep, "identity": identity,
    }
    in_maps = []
    for core in range(NCORES):
        senders16 = np.zeros((P, gtot * 8), np.int16)
        efTa = np.zeros((edge_dim + 1, ecap), BF)
        efTa[edge_dim, :] = 1.0
        rl_flat = np.full(ecap, int(PAD_RLOC), np.int64)
        blocknodes = np.zeros((P, npos), np.int32)
        for pos in range(npos):
            b = int(plan["blocks"][pos, core])
            g0 = int(plan["goff"][pos])
            gp = int(plan["gpos"][pos])
            c = int(plan["cnt"][b])
            e0 = int(plan["estart"][b])
            blocknodes[:, pos] = b * P + np.arange(P)
            col0 = g0 * P
            efTa[:edge_dim, col0:col0 + c] = efT_all[:, e0:e0 + c]
            rl_flat[col0:col0 + c] = rloc_all[e0:e0 + c]
            for (g_lo, ng, base) in plan["gdesc"][pos]:
                s_ch = ng * P
                tmp_s = np.full(s_ch, base, np.int64)  # pads -> row `base`
                r0 = g_lo * P
                nreal = min(max(c - r0, 0), s_ch)
                tmp_s[:nreal] = s_sorted[e0 + r0:e0 + r0 + nreal]
                rel = (tmp_s - base).astype(np.int16)
                blk16 = np.tile(rel.reshape(s_ch // 16, 16).T, (8, 1))
                cb = (g0 * P + r0) // 16
                senders16[:, cb:cb + s_ch // 16] = blk16
        # receiver one-hots, both orientations, fp8 (pads -> all-zero),
        # packed per position as [GTh_pos | Gtf_pos] for one DMA each
        GTh = (rl_flat[None, :] == qrange[:, None])
        rl_g = rl_flat.reshape(gtot, P)  # [g, p] -> rloc of edge g*128+p
        Gtf = (rl_g[:, :, None] == qrange[None, None, :])  # [g, p, q]
        GG = np.zeros((P, 2 * ecap), F8)
        for pos in range(npos):
            g0 = int(plan["goff"][pos])
            gp = int(plan["gpos"][pos])
            for cc in range(-(-gp // CHUNK_G)):
                gc_ = min(CHUNK_G, gp - cc * CHUNK_G)
                sc_ = gc_ * P
                gl = g0 + cc * CHUNK_G
                c0 = 2 * (g0 * P + cc * CHUNK_G * P)
                GG[:, c0:c0 + sc_] = \
                    GTh[:, gl * P:gl * P + sc_].astype(F8)
                GG[:, c0 + sc_:c0 + 2 * sc_] = np.ascontiguousarray(
                    Gtf[gl:gl + gc_].transpose(1, 0, 2).reshape(P, sc_)
                ).astype(F8)
        m = dict(shared)
        m.update({"senders16": senders16, "efTa": efTa,
                  "GG": GG, "blocknodes": blocknodes,
                  "nfT2": (nfT2_cores[core] if PHASE_A_MODE == "allgather"
                           else nfT2)})
        in_maps.append(m)
    return in_maps


# ---------------------------------------------------------------- bass build

def _build(plan, n_pad, in_dim, edge_dim, embed, heads, debug=False,
           repeat=1, parts="full"):
    head_dim = embed // heads
    npos, gtot, ecap = plan["npos"], plan["gtot"], plan["ecap"]
    gpos, goff = plan["gpos"], plan["goff"]
    gpmax = int(gpos.max())
    UW = embed + heads  # U columns: [weighted sum | denom]

    nc = bacc.Bacc("TRN2", num_swdge_queues=4,
                   dynamic_dma_scratch_size=65536)
    SH = n_pad // NCORES
    nfT2_cols = 2 * SH if PHASE_A_MODE == "allgather" else 2 * n_pad
    t_nfT2 = nc.dram_tensor("nfT2", [P, nfT2_cols], BF16,
                            kind="ExternalInput")
    t_W = nc.dram_tensor("W", [in_dim, embed], BF16, kind="ExternalInput")
    t_We = nc.dram_tensor("We_aug", [edge_dim + 1, embed], BF16,
                          kind="ExternalInput")
    t_A = nc.dram_tensor("A_blk", [embed, heads], BF16, kind="ExternalInput")
    t_A2 = nc.dram_tensor("Abm2", [embed, heads], BF16, kind="ExternalInput")
    t_Wb = nc.dram_tensor("Wb_rep", [P, embed], F32, kind="ExternalInput")
    t_id = nc.dram_tensor("identity", [P, P], BF16, kind="ExternalInput")
    t_s16 = nc.dram_tensor("senders16", [P, gtot * 8], mybir.dt.int16,
                           kind="ExternalInput")
    t_efT = nc.dram_tensor("efTa", [edge_dim + 1, ecap], BF16,
                           kind="ExternalInput")
    t_GG = nc.dram_tensor("GG", [P, 2 * ecap], FP8, kind="ExternalInput")
    t_bn = nc.dram_tensor("blocknodes", [P, npos], I32, kind="ExternalInput")
    t_out = nc.dram_tensor("out", [npos * P, embed], F32,
                           kind="ExternalOutput")
    t_h = nc.dram_tensor("h_scratch", [n_pad, embed], BF16, kind="Internal")

    with tile.TileContext(nc) as tc:
        with tc.tile_pool(name="const", bufs=1) as cp:
            def cload(t, shape):
                s = cp.tile(shape, t.dtype, tag=f"c_{t.name}")
                nc.sync.dma_start(out=s[:], in_=t[:])
                return s

            W0 = cp.tile([P, embed], BF16)
            nc.sync.dma_start(out=W0[:], in_=t_W[0:P, :])
            W1 = cp.tile([P, embed], BF16)
            nc.sync.dma_start(out=W1[:], in_=t_W[P:2 * P, :])
            We = cload(t_We, [edge_dim + 1, embed])
            Ab = cload(t_A, [embed, heads])
            Abm2 = cload(t_A2, [embed, heads])
            Wb = cload(t_Wb, [P, embed])
            idn = cload(t_id, [P, P])
            s16 = cload(t_s16, [P, gtot * 8])
            bn = cload(t_bn, [P, npos])

            # ---------------- phase A: h = nf @ W (no bias) ----------------
            if PHASE_A_MODE == "allgather":
                t_hs = nc.dram_tensor("h_shard", [SH, embed], BF16,
                                      kind="Internal")
                a_rows, a_dst, HW_ = SH, t_hs, 7 * P
            else:
                a_rows, a_dst, HW_ = n_pad, t_h, 8 * P
            for _rep in range(repeat):
              with tc.tile_pool(name=f"ha{_rep}", bufs=4) as hap, \
                      tc.tile_pool(name=f"haps{_rep}", bufs=3, space="PSUM") as hpp:
                  for nt in range(a_rows // HW_):
                      na = hap.tile([P, 2 * HW_], BF16, tag="nfT0")
                      nc.sync.dma_start(
                          out=na[:],
                          in_=t_nfT2[:, 2 * nt * HW_:2 * (nt + 1) * HW_])
                      hstage = hap.tile([P, HW_], BF16, tag="hstage")
                      hp = hpp.tile([P, HW_], F32, tag="hps")
                      for t in range(HW_ // P):
                          nc.tensor.matmul(hp[:, t * P:(t + 1) * P],
                                           lhsT=na[:, t * P:(t + 1) * P],
                                           rhs=W0[:], start=True, stop=False)
                          nc.tensor.matmul(hp[:, t * P:(t + 1) * P],
                                           lhsT=na[:, HW_ + t * P:
                                                    HW_ + (t + 1) * P],
                                           rhs=W1[:], start=False, stop=True)
                      nc.scalar.activation(out=hstage[:], in_=hp[:],
                                           func=AF.Copy)
                      out_view = bass.AP(
                          a_dst[:].tensor, nt * HW_ * embed,
                          [[embed, P], [P * embed, HW_ // P], [1, embed]])
                      nc.scalar.dma_start(out=out_view, in_=hstage[:])

              tc.strict_bb_all_engine_barrier()
              if PHASE_A_MODE == "allgather":
                  nc.gpsimd.collective_compute(
                      "AllGather", mybir.AluOpType.bypass,
                      replica_groups=[list(range(NCORES))],
                      ins=[t_hs[:].opt()], outs=[t_h[:].opt()])
                  tc.strict_bb_all_engine_barrier()
              if parts == "a":
                  continue

              # ---------------- phase B: edge processing ---------------------
              # Positions run in interleaved groups of G_ILV purely for
              # cross-engine pipelining (stage 1 of the whole group issues
              # before stage 2/3, so no engine queue stalls on another
              # engine's tail).
              G_ILV = 3
              # fraction of chunks whose softmax-weight expansion runs as an
              # ACT broadcast-copy (DVE mult then runs at 2x) instead of a
              # DVE broadcast-mult; balances the ACT and DVE engines.
              MV_ACT_FRAC = 1.0
              with tc.tile_pool(name=f"eb{_rep}", bufs=6) as ep, \
                      tc.tile_pool(name=f"ebsm{_rep}", bufs=3) as esm, \
                      tc.tile_pool(name=f"ebp{_rep}", bufs=2, space="PSUM") as pp, \
                      tc.tile_pool(name=f"lgp{_rep}", bufs=2, space="PSUM") as lp, \
                      tc.tile_pool(name=f"ups{_rep}", bufs=2, space="PSUM") as up:
                  nd2_tiles = []
                  qi = 0  # gather queue rotation (1..3; 0 = indirect DMAs)
                  mv_acc = 0.0
                  for p0 in range(0, npos, G_ILV):
                      grp = list(range(p0, min(p0 + G_ILV, npos)))
                      # issue output stores lagged by 2 groups: the data is
                      # long since ready, so the SP queue never blocks on it
                      while len(nd2_tiles) > 2 * G_ILV:
                          pos_, t_ = nd2_tiles.pop(0)
                          nc.sync.dma_start(
                              out=t_out[pos_ * P:(pos_ + 1) * P, :],
                              in_=t_[:])
                      st = {}
                      UpsG = up.tile([P, G_ILV * UW], F32, tag="U", bufs=1)
                      # --- stage 1: x into PSUM, u = e^x, rc, q, logits ------
                      for pos in grp:
                          g_here = int(gpos[pos])
                          g0 = int(goff[pos])
                          Hb = ep.tile([P, embed], BF16, tag="Hb")
                          nc.gpsimd.indirect_dma_start(
                              out=Hb[:], out_offset=None, in_=t_h[:],
                              in_offset=bass.IndirectOffsetOnAxis(
                                  ap=bn[:, pos:pos + 1], axis=0))
                          k_ = pos - p0
                          Ups = UpsG[:, k_ * UW:(k_ + 1) * UW]
                          lgb = lp.tile([P, ((gpmax * heads + 127) // 128)
                                         * 128], F32, tag="lgb")
                          nchunks = -(-g_here // CHUNK_G)
                          d = dict(g_here=g_here, g0=g0, ggc=[], Ups=Ups,
                                   lgb=lgb, nchunks=nchunks, es=[])
                          st[pos] = d
                          for c in range(nchunks):
                              gc = min(CHUNK_G, g_here - c * CHUNK_G)
                              s = gc * P
                              co = c * CHUNK_G * P
                              es = ep.tile([P, CHUNK_G * P], BF16, tag="es",
                                           bufs=11)
                              d["es"].append(es)
                              ggc = ep.tile([P, 2 * CHUNK_G * P], FP8,
                                            tag="ggc", bufs=10)
                              d["ggc"].append(ggc)
                              nc.sync.dma_start(
                                  out=ggc[:, :2 * s],
                                  in_=t_GG[:, 2 * (g0 * P + co):
                                           2 * (g0 * P + co) + 2 * s])
                              efc = ep.tile([edge_dim + 1, CHUNK_G * P], BF16,
                                            tag="efc", bufs=6)
                              nc.sync.dma_start(
                                  out=efc[:, :s],
                                  in_=t_efT[:, g0 * P + co:g0 * P + co + s])
                              for (g_lo, ng, base) in plan["gdesc"][pos]:
                                  if not (c * CHUNK_G <= g_lo
                                          < c * CHUNK_G + gc):
                                      continue
                                  rows = min(n_pad - base, 32768)
                                  cb = g0 * 8 + g_lo * 8
                                  o0 = (g_lo - c * CHUNK_G) * P
                                  sg = ng * P
                                  nc.gpsimd.dma_gather(
                                      out_ap=es[:, o0:o0 + sg].rearrange(
                                          "p (j e) -> p j e", e=embed),
                                      in_ap=t_h[base:base + rows, :],
                                      idxs_ap=s16[:, cb:cb + sg // 16],
                                      num_idxs=sg, num_idxs_reg=sg,
                                      elem_size=embed,
                                      queue_num=1 + qi % 3)
                                  qi += 1
                              if parts == "ag":
                                  continue
                              at = pp.tile([P, CHUNK_G * P], F32, tag="attnT",
                                           bufs=2)
                              for o_ in range(0, s, 512):
                                  w_ = min(512, s - o_)
                                  nc.tensor.matmul(at[:, o_:o_ + w_],
                                                   lhsT=We[:],
                                                   rhs=efc[:, o_:o_ + w_],
                                                   start=True, stop=False)
                                  nc.tensor.matmul(at[:, o_:o_ + w_],
                                                   lhsT=Hb[:],
                                                   rhs=ggc[:, o_:o_ + w_],
                                                   start=False, stop=False)
                              for j in range(gc):
                                  # es_j^T via regular matmul against identity
                                  # (is_transpose would force bf16 PSUM out).
                                  # stop closes each 2KB PSUM zero-region's
                                  # accumulation group (last j per region).
                                  nc.tensor.matmul(
                                      at[:, j * P:(j + 1) * P],
                                      lhsT=es[:, j * P:(j + 1) * P],
                                      rhs=idn[:],
                                      start=False,
                                      stop=(j % 4 == 3 or j == gc - 1))
                              # mish(x) = x*(1 - 2/((1+e^x)^2+1)); the -2
                              # folds into the Abm2 logits matmul, so only
                              # u = e^x, rc = 1/((1+u)^2+1) and q = x*rc are
                              # materialized.
                              # (the fused ANT_MISH1_MUL variant that reads x
                              # straight from PSUM was tried and is SLOWER on
                              # HW — 541us vs 422us replicated — because the
                              # at PSUM tile then stays live through the
                              # ACT->DVE->DVE chain and PE stalls on the 2
                              # PSUM buffers.  The xc Copy frees at early.)
                              u = ep.tile([P, CHUNK_G * P], BF16, tag="u",
                                          bufs=4)
                              nc.scalar.activation(out=u[:, :s],
                                                   in_=at[:, :s], func=AF.Exp)
                              xc = ep.tile([P, CHUNK_G * P], BF16, tag="xc",
                                           bufs=4)
                              nc.scalar.activation(out=xc[:, :s],
                                                   in_=at[:, :s],
                                                   func=AF.Copy)
                              den = ep.tile([P, CHUNK_G * P], BF16, tag="den",
                                            bufs=4)
                              nc.vector._custom_dve(
                                  _GAT_DEN, out=den[:, :s], in0=u[:, :s],
                                  s0=1.0)
                              q = ep.tile([P, CHUNK_G * P], BF16, tag="q",
                                          bufs=4)
                              nc.vector._custom_dve(
                                  _RECIP1_MUL, out=q[:, :s],
                                  in0=den[:, :s], in1=xc[:, :s], **RECIP1_C)
                              for j in range(gc):
                                  jo = (c * CHUNK_G + j) * heads
                                  nc.tensor.matmul(
                                      lgb[:, jo:jo + heads],
                                      lhsT=xc[:, j * P:(j + 1) * P],
                                      rhs=Ab[:], start=True, stop=False)
                                  nc.tensor.matmul(
                                      lgb[:, jo:jo + heads],
                                      lhsT=q[:, j * P:(j + 1) * P],
                                      rhs=Abm2[:], start=False, stop=True)
                      if parts == "ag":
                          continue
                      # --- stage 2: softmax numerators -----------------------
                      for pos in grp:
                          d = st[pos]
                          g_here = d["g_here"]
                          exb = esm.tile([P, gpmax * heads], BF16, tag="exb")
                          d["exb"] = exb
                          nc.scalar.activation(
                              out=exb[:, :g_here * heads],
                              in_=d["lgb"][:, :g_here * heads],
                              func=AF.Exp)
                          # optional ACT-side expansion of the per-head
                          # weights to per-head-dim (stage 3's mult then runs
                          # without a broadcast operand, i.e. at 2x)
                          d["exw"] = []
                          for c in range(d["nchunks"]):
                              gc = min(CHUNK_G, g_here - c * CHUNK_G)
                              mv_acc += MV_ACT_FRAC
                              if mv_acc >= 1.0:
                                  mv_acc -= 1.0
                                  exw = esm.tile([P, CHUNK_G * P], BF16,
                                                 tag="exw", bufs=10)
                                  src = exb[:, c * CHUNK_G * heads:
                                            (c * CHUNK_G + gc) * heads]
                                  src_b = src.rearrange(
                                      "p (j h) -> p j h",
                                      j=gc).to_broadcast(
                                      [P, gc, heads, head_dim])
                                  dst = exw[:, :gc * P].rearrange(
                                      "p (j h w) -> p j h w", j=gc,
                                      w=head_dim)
                                  nc.scalar.activation(out=dst, in_=src_b,
                                                       func=AF.Copy)
                                  d["exw"].append(exw)
                              else:
                                  d["exw"].append(None)
                      # --- stage 3: weighted scatter-accumulate --------------
                      for pos in grp:
                          d = st[pos]
                          g_here = d["g_here"]
                          exb = d["exb"]
                          Ups = d["Ups"]
                          nchunks = d["nchunks"]
                          for c in range(nchunks):
                              gc = min(CHUNK_G, g_here - c * CHUNK_G)
                              s = gc * P
                              es = d["es"][c]
                              rb = ep.tile([P, CHUNK_G * UW], BF16,
                                           tag="rhsb", bufs=4)
                              rb3 = rb[:].rearrange("p (j c) -> p j c",
                                                    j=CHUNK_G)
                              ex_view = rb3[:, :gc, embed:UW]
                              exb_view = exb[:, c * CHUNK_G * heads:
                                             (c * CHUNK_G + gc) *
                                             heads].rearrange(
                                  "p (j h) -> p j h", j=gc)
                              nc.vector.tensor_copy(out=ex_view, in_=exb_view)
                              m_view = rb3[:, :gc, 0:embed].rearrange(
                                  "p j (h w) -> p j h w", w=head_dim)
                              es_view = es[:, :s].rearrange(
                                  "p (j h w) -> p j h w", j=gc, w=head_dim)
                              exw = d["exw"][c]
                              if exw is not None:
                                  exw_view = exw[:, :s].rearrange(
                                      "p (j h w) -> p j h w", j=gc,
                                      w=head_dim)
                                  nc.vector.tensor_tensor(
                                      out=m_view, in0=es_view, in1=exw_view,
                                      op=OP.mult)
                              else:
                                  ex_b = exb_view.to_broadcast(
                                      [P, gc, heads, head_dim])
                                  nc.vector.tensor_tensor(
                                      out=m_view, in0=es_view, in1=ex_b,
                                      op=OP.mult)
                              for j in range(gc):
                                  nc.tensor.matmul(
                                      Ups[:],
                                      lhsT=d["ggc"][c][
                                          :, s + j * P:s + (j + 1) * P],
                                      rhs=rb[:, j * UW:(j + 1) * UW],
                                      start=(c == 0 and j == 0),
                                      stop=(c == nchunks - 1 and j == gc - 1))
                          # -- block epilogue: out = U / max(denom, eps) + Wb --
                          dn = ep.tile([P, heads], F32, tag="dn")
                          nc.vector.tensor_scalar(out=dn[:],
                                                  in0=Ups[:, embed:UW],
                                                  scalar1=1e-30, scalar2=None,
                                                  op0=OP.max)
                          rc = ep.tile([P, heads], F32, tag="rc")
                          nc.vector.reciprocal(rc[:], dn[:])
                          nd = ep.tile([P, embed], F32, tag="nodes")
                          ndv = nd[:].rearrange("p (h w) -> p h w", w=head_dim)
                          uv = Ups[:, 0:embed].rearrange("p (h w) -> p h w",
                                                         w=head_dim)
                          rcb = rc[:].to_broadcast([P, heads, head_dim])
                          nc.vector.tensor_tensor(out=ndv, in0=uv, in1=rcb,
                                                  op=OP.mult)
                          nd2 = ep.tile([P, embed], F32, tag="nodes2",
                                        bufs=8)
                          nc.vector.tensor_tensor(out=nd2[:], in0=nd[:],
                                                  in1=Wb[:], op=OP.add)
                          nd2_tiles.append((pos, nd2))
                  # deferred output stores: keep the SP queue free of
                  # pipeline-dependent stores during the main loop
                  for pos_, t_ in nd2_tiles:
                      nc.sync.dma_start(
                          out=t_out[pos_ * P:(pos_ + 1) * P, :], in_=t_[:])
    nc.finalize()
    return nc


# ---------------------------------------------------------------- entry

def _run(node_features, edge_features, W_kernel, W_bias, We_kernel, We_bias,
         a, senders, receivers, trace=False):
    n_nodes, in_dim = node_features.shape
    heads, head_dim = a.shape
    embed = heads * head_dim
    edge_dim = edge_features.shape[1]
    plan = _plan(receivers, senders, n_nodes, NCORES)
    n_pad = plan["nb_pad"] * P
    in_maps = _host_inputs(plan, node_features, edge_features, W_kernel,
                           W_bias, We_kernel, We_bias, a, senders)
    nc = _build(plan, n_pad, in_dim, edge_dim, embed, heads)
    res = run_bass_kernel_spmd(nc, in_maps, core_ids=list(range(NCORES)),
                               trace=trace)
    # reassemble: core outputs are [npos*P, embed]; position rows -> blocks
    out = np.zeros((n_pad, embed), np.float32)
    for core in range(NCORES):
        o = res.results[core]["out"]
        for pos in range(plan["npos"]):
            b = int(plan["blocks"][pos, core])
            out[b * P:(b + 1) * P] = o[pos * P:(pos + 1) * P]
    out = out[:n_nodes]
    # nodes with no incoming edges: reference segment_sum gives exactly 0
    deg = np.bincount(receivers.astype(np.int64), minlength=n_nodes)
    if (deg == 0).any():
        out[deg == 0] = 0.0
    return out, res


def kernel(node_features, edge_features, W_kernel, W_bias, We_kernel,
           We_bias, a, senders, receivers):
    node_features = np.asarray(node_features, np.float32)
    edge_features = np.asarray(edge_features, np.float32)
    W_kernel = np.asarray(W_kernel, np.float32)
    W_bias = np.asarray(W_bias, np.float32)
    We_kernel = np.asarray(We_kernel, np.float32)
    We_bias = np.asarray(We_bias, np.float32)
    a = np.asarray(a, np.float32)
    senders = np.asarray(senders, np.int32)
    receivers = np.asarray(receivers, np.int32)
    out, _ = _run(node_features, edge_features, W_kernel, W_bias, We_kernel,
                  We_bias, a, senders, receivers)
    return out

